# revision 7
# baseline (speedup 1.0000x reference)
"""Trainium2 Bass kernel for greedy GRU decode (AnswerModule).

B=64, H=1024, V=50257 (padded 51200), T=20 steps, 8 NeuronCores.

Strategy (tensor-parallel over vocab):
 - W_out/b_out/word-emb-gather sharded over vocab (6400 rows/core).
 - Screen: bf16 matmul h @ W_out_shard.T (+bias via K=1 matmul row) -> fp32 psum.
 - top-8 via max8/max_index; top-4 rescored exactly in fp32 via indirect-DMA
   gather of [W|b] rows + tensor_tensor_reduce dots.
 - AllGather (val,idx) -> global argmax with lowest-index tie-break.
 - Embedding gather from a replicated table; GRU sharded over H (128 rows/core)
   in fp32; AllGather h chunks.
"""
import sys
import numpy as np

sys.path.insert(0, "/opt/trn_rl_repo")
sys.path.insert(0, "/root/.axon_site")

import ml_dtypes

B = 64
H = 1024
V = 50257
VPAD = 51200
VSH = VPAD // 8          # 6400
T = 20
NCORES = 8
NK = H // 128            # 8 contraction chunks
NV = VSH // 512          # 12.5 -> handle as 12 full + 1 half? use 400-col tiles instead
# use vtile size 512 with 12 full tiles + 1 tile of 256: 12*512+256 = 6400
VT_SIZES = [512] * 12 + [256]
KCAND = 4
BIG = float(1 << 24)
PAD_BIAS = -10000.0


def build(steps=T):
    import concourse.bass as bass
    import concourse.bacc as bacc
    import concourse.mybir as mybir
    from concourse import tile
    from concourse.tile_rust import add_dep_helper
    from concourse.masks import make_identity

    F32 = mybir.dt.float32
    BF16 = mybir.dt.bfloat16
    U32 = mybir.dt.uint32
    I32 = mybir.dt.int32
    AF = mybir.ActivationFunctionType
    ALU = mybir.AluOpType
    AX = mybir.AxisListType

    nc = bacc.Bacc("TRN2", target_bir_lowering=False, debug=False, num_devices=NCORES)

    # ---- external inputs (per-core shards prepared on host) ----
    wt_bf = nc.dram_tensor("wt_bf", [128, NK, VSH], BF16, kind="ExternalInput")
    bias_bf = nc.dram_tensor("bias_bf", [1, VSH], BF16, kind="ExternalInput")
    w_aug = nc.dram_tensor("w_aug", [VSH, 1032], F32, kind="ExternalInput")
    wemb = nc.dram_tensor("wemb", [V, 1024], F32, kind="ExternalInput")
    we_lhsT = nc.dram_tensor("we_lhsT", [128, 3, NK, 128], F32, kind="ExternalInput")
    whh_lhsT = nc.dram_tensor("whh_lhsT", [128, 3, NK, 128], F32, kind="ExternalInput")
    cT_in = nc.dram_tensor("cT_in", [128, 3, 64], F32, kind="ExternalInput")
    bhh_n_in = nc.dram_tensor("bhh_n_in", [128, 1], F32, kind="ExternalInput")
    h0_own_in = nc.dram_tensor("h0_own_in", [128, 64], F32, kind="ExternalInput")
    hT0_in = nc.dram_tensor("hT0_in", [128, NK, 64], F32, kind="ExternalInput")
    haug0_in = nc.dram_tensor("haug0_in", [64, 1032], F32, kind="ExternalInput")
    coff_in = nc.dram_tensor("coff_in", [64, 1], F32, kind="ExternalInput")

    out = nc.dram_tensor("out", [64, steps], I32, kind="ExternalOutput")

    # ---- collective DRAM buffers (double buffered) ----
    ag1_in = [nc.dram_tensor(f"ag1_in{i}", [64, 2], F32) for i in range(2)]
    ag1_out = [nc.dram_tensor(f"ag1_out{i}", [8, 64, 2], F32, addr_space="Shared") for i in range(2)]
    ag2_in = [nc.dram_tensor(f"ag2_in{i}", [128, 64], F32) for i in range(2)]
    ag2_out = [nc.dram_tensor(f"ag2_out{i}", [8, 128, 64], F32, addr_space="Shared") for i in range(2)]

    from contextlib import ExitStack
    ctx = ExitStack()
    with ctx:
        tc = ctx.enter_context(tile.TileContext(nc))

        # ---- sbuf tensors ----
        wt_sb = nc.alloc_sbuf_tensor("wt_sb", [128, NK, VSH], BF16)
        bias_sb = nc.alloc_sbuf_tensor("bias_sb", [1, VSH], BF16)
        ones_sb = nc.alloc_sbuf_tensor("ones_sb", [1, 64], BF16)
        we_sb = nc.alloc_sbuf_tensor("we_sb", [128, 3, NK, 128], F32)
        whh_sb = nc.alloc_sbuf_tensor("whh_sb", [128, 3, NK, 128], F32)
        cT_sb = nc.alloc_sbuf_tensor("cT_sb", [128, 3, 64], F32)
        bhhn_sb = nc.alloc_sbuf_tensor("bhhn_sb", [128, 1], F32)
        coff_sb = nc.alloc_sbuf_tensor("coff_sb", [64, 1], F32)
        ident64 = nc.alloc_sbuf_tensor("ident64", [64, 64], F32)
        ident128 = nc.alloc_sbuf_tensor("ident128", [128, 128], F32)

        hT = nc.alloc_sbuf_tensor("hT", [128, NK, 64], F32)
        hT_bf = nc.alloc_sbuf_tensor("hT_bf", [128, NK, 64], BF16)
        h_aug = nc.alloc_sbuf_tensor("h_aug", [64, 1032], F32)
        h_own = nc.alloc_sbuf_tensor("h_own", [128, 64], F32)
        hnew = nc.alloc_sbuf_tensor("hnew", [128, 64], F32)
        embT = nc.alloc_sbuf_tensor("embT", [128, NK, 64], F32)
        emb_sb = nc.alloc_sbuf_tensor("emb_sb", [64, 1024], F32)

        logits = nc.alloc_sbuf_tensor("logits", [64, VSH], F32)
        maxv = nc.alloc_sbuf_tensor("maxv", [64, 8], F32)
        maxi = nc.alloc_sbuf_tensor("maxi", [64, 8], U32)
        maxi_f = nc.alloc_sbuf_tensor("maxi_f", [64, KCAND], F32)
        g4 = nc.alloc_sbuf_tensor("g4", [64, KCAND, 1032], F32)
        resc = nc.alloc_sbuf_tensor("resc", [64, KCAND], F32)

        rmax = nc.alloc_sbuf_tensor("rmax", [64, 1], F32)
        rtmp = nc.alloc_sbuf_tensor("rtmp", [64, KCAND], F32)
        rmask = nc.alloc_sbuf_tensor("rmask", [64, KCAND], F32)
        lidx = nc.alloc_sbuf_tensor("lidx", [64, 1], F32)
        agin_sb = nc.alloc_sbuf_tensor("agin_sb", [64, 2], F32)
        gg = nc.alloc_sbuf_tensor("gg", [64, 8, 2], F32)
        gmax = nc.alloc_sbuf_tensor("gmax", [64, 1], F32)
        gmask = nc.alloc_sbuf_tensor("gmask", [64, 8], F32)
        gtmp = nc.alloc_sbuf_tensor("gtmp", [64, 8], F32)
        tokf = nc.alloc_sbuf_tensor("tokf", [64, 1], F32)
        toku = nc.alloc_sbuf_tensor("toku", [64, 1], U32)
        toks = nc.alloc_sbuf_tensor("toks", [64, steps], I32)

        r_sb = nc.alloc_sbuf_tensor("r_sb", [128, 64], F32)
        z_sb = nc.alloc_sbuf_tensor("z_sb", [128, 64], F32)
        n_sb = nc.alloc_sbuf_tensor("n_sb", [128, 64], F32)
        gt1 = nc.alloc_sbuf_tensor("gt1", [128, 64], F32)
        gt2 = nc.alloc_sbuf_tensor("gt2", [128, 64], F32)

        # ---- psum ----
        ps_scr = [ctx.enter_context(nc.psum_tensor(f"ps_scr{i}", [64, 512], F32)) for i in range(2)]
        ps_g = ctx.enter_context(nc.psum_tensor("ps_g", [128, 2, 64], F32))
        ps_ghn = ctx.enter_context(nc.psum_tensor("ps_ghn", [128, 64], F32))
        ps_gin = ctx.enter_context(nc.psum_tensor("ps_gin", [128, 64], F32))
        ps_e = ctx.enter_context(nc.psum_tensor("ps_e", [128, 512], F32))
        ps_h0 = ctx.enter_context(nc.psum_tensor("ps_h0", [64, 512], F32))
        ps_h1 = ctx.enter_context(nc.psum_tensor("ps_h1", [64, 512], F32))

        # ---- preamble: load everything ----
        nc.sync.dma_start(wt_sb[:], wt_bf[:])
        nc.sync.dma_start(bias_sb[:], bias_bf[:])
        nc.sync.dma_start(we_sb[:], we_lhsT[:])
        nc.sync.dma_start(whh_sb[:], whh_lhsT[:])
        nc.sync.dma_start(cT_sb[:], cT_in[:])
        nc.sync.dma_start(bhhn_sb[:], bhh_n_in[:])
        nc.sync.dma_start(coff_sb[:], coff_in[:])
        nc.sync.dma_start(h_own[:], h0_own_in[:])
        nc.sync.dma_start(hT[:], hT0_in[:])
        nc.sync.dma_start(h_aug[:], haug0_in[:])
        nc.vector.tensor_copy(hT_bf[:], hT[:])
        nc.vector.memset(ones_sb[:], 1.0)
        make_identity(nc, ident64[:])
        make_identity(nc, ident128[:])

        prev_gg_read = [None, None]   # for WAR dep two steps back (ag1)
        prev_hT_read = [None, None]   # (ag2)

        for t in range(steps):
            db = t % 2

            # ===== screen matmuls (bf16) + bias row =====
            voff = 0
            for vt, vsz in enumerate(VT_SIZES):
                ps = ps_scr[vt % 2]
                for k in range(NK):
                    nc.tensor.matmul(
                        ps[:, 0:vsz],
                        hT_bf[:, k, :],
                        wt_sb[:, k, voff:voff + vsz],
                        start=(k == 0), stop=False)
                nc.tensor.matmul(
                    ps[:, 0:vsz],
                    ones_sb[:],
                    bias_sb[:, voff:voff + vsz],
                    start=False, stop=True)
                nc.scalar.copy(logits[:, voff:voff + vsz], ps[:, 0:vsz])
                voff += vsz

            # ===== local top-8 =====
            nc.vector.max(out=maxv[:], in_=logits[:])
            nc.vector.max_index(out=maxi[:], in_max=maxv[:], in_values=logits[:])
            nc.vector.tensor_copy(maxi_f[:], maxi[:, 0:KCAND])

            # ===== gather candidate [W|b] rows + exact rescore =====
            for j in range(KCAND):
                nc.gpsimd.indirect_dma_start(
                    out=g4[:, j, :],
                    out_offset=None,
                    in_=w_aug[:],
                    in_offset=bass.IndirectOffsetOnAxis(ap=maxi[:, j:j + 1], axis=0),
                )
            nc.vector.tensor_mul(
                g4[:], g4[:],
                h_aug[:].unsqueeze(1).to_broadcast([64, KCAND, 1032]))
            nc.vector.tensor_reduce(resc[:], g4[:], axis=AX.X, op=ALU.add)

            # ===== local argmax of rescored (lowest global idx on ties) =====
            nc.vector.tensor_reduce(rmax[:], resc[:], axis=AX.X, op=ALU.max)
            nc.vector.tensor_scalar(rmask[:], resc[:], rmax[:, 0:1], None, op0=ALU.is_equal)
            nc.vector.tensor_scalar_add(rtmp[:], maxi_f[:], coff_sb[:, 0:1])   # global idx
            nc.vector.tensor_scalar_add(rtmp[:], rtmp[:], -BIG)
            nc.vector.tensor_mul(rtmp[:], rtmp[:], rmask[:])
            nc.vector.tensor_scalar_add(rtmp[:], rtmp[:], BIG)
            nc.vector.tensor_reduce(lidx[:], rtmp[:], axis=AX.X, op=ALU.min)
            nc.vector.tensor_copy(agin_sb[:, 0:1], rmax[:])
            nc.vector.tensor_copy(agin_sb[:, 1:2], lidx[:])

            # ===== AllGather candidates =====
            w1 = nc.sync.dma_start(ag1_in[db][:], agin_sb[:])
            cc1 = nc.gpsimd.collective_compute(
                "AllGather", ALU.bypass,
                replica_groups=[list(range(NCORES))],
                ins=[ag1_in[db][:]], outs=[ag1_out[db][:]],
            )
            add_dep_helper(cc1.ins, w1.ins, True, "ag1 after input write")
            if prev_gg_read[db] is not None:
                add_dep_helper(cc1.ins, prev_gg_read[db].ins, True, "ag1 WAR")
            r1 = nc.sync.dma_start(
                gg[:],
                bass.AP(ag1_out[db], 0, [[2, 64], [128, 8], [1, 2]]),
            )
            add_dep_helper(r1.ins, cc1.ins, True, "gg read after ag1")
            prev_gg_read[db] = r1

            # ===== global argmax combine =====
            nc.vector.tensor_reduce(gmax[:], gg[:, :, 0], axis=AX.X, op=ALU.max)
            nc.vector.tensor_scalar(gmask[:], gg[:, :, 0], gmax[:, 0:1], None, op0=ALU.is_equal)
            nc.vector.tensor_scalar_add(gtmp[:], gg[:, :, 1], -BIG)
            nc.vector.tensor_mul(gtmp[:], gtmp[:], gmask[:])
            nc.vector.tensor_scalar_add(gtmp[:], gtmp[:], BIG)
            nc.vector.tensor_reduce(tokf[:], gtmp[:], axis=AX.X, op=ALU.min)
            nc.vector.tensor_copy(toku[:], tokf[:])
            nc.vector.tensor_copy(toks[:, t:t + 1], tokf[:])

            # ===== embedding gather + transpose =====
            nc.gpsimd.indirect_dma_start(
                out=emb_sb[:],
                out_offset=None,
                in_=wemb[:],
                in_offset=bass.IndirectOffsetOnAxis(ap=toku[:, 0:1], axis=0),
            )
            for k in range(NK):
                nc.tensor.transpose(ps_e[:, k * 64:(k + 1) * 64],
                                    emb_sb[:, k * 128:(k + 1) * 128], ident64[:])
                nc.scalar.copy(embT[:, k, :], ps_e[:, k * 64:(k + 1) * 64])

            # ===== GRU matmuls =====
            # r/z gates: gi+gh merged into one psum accumulation (16 MMs each)
            for g in range(2):
                for k in range(NK):
                    nc.tensor.matmul(
                        ps_g[:, g, :], whh_sb[:, g, k, :], hT[:, k, :],
                        start=(k == 0), stop=False)
                for k in range(NK):
                    nc.tensor.matmul(
                        ps_g[:, g, :], we_sb[:, g, k, :], embT[:, k, :],
                        start=False, stop=(k == NK - 1))
            # n gate: gh and gi kept separate (gh_n is scaled by r)
            for k in range(NK):
                nc.tensor.matmul(
                    ps_ghn[:], whh_sb[:, 2, k, :], hT[:, k, :],
                    start=(k == 0), stop=(k == NK - 1))
            for k in range(NK):
                nc.tensor.matmul(
                    ps_gin[:], we_sb[:, 2, k, :], embT[:, k, :],
                    start=(k == 0), stop=(k == NK - 1))

            # ===== gates =====
            # r = sigmoid(gi_r + gh_r + c_r)  via exp/recip
            nc.vector.tensor_add(gt1[:], ps_g[:, 0, :], cT_sb[:, 0, :])
            nc.scalar.activation(gt2[:], gt1[:], AF.Exp, scale=-1.0)
            nc.vector.tensor_scalar_add(gt2[:], gt2[:], 1.0)
            nc.vector.reciprocal(r_sb[:], gt2[:])
            # z
            nc.vector.tensor_add(gt1[:], ps_g[:, 1, :], cT_sb[:, 1, :])
            nc.scalar.activation(gt2[:], gt1[:], AF.Exp, scale=-1.0)
            nc.vector.tensor_scalar_add(gt2[:], gt2[:], 1.0)
            nc.vector.reciprocal(z_sb[:], gt2[:])
            # n = tanh(gi_n + c_n + r * (gh_n + bhh_n))
            nc.vector.tensor_scalar_add(gt1[:], ps_ghn[:], bhhn_sb[:, 0:1])
            nc.vector.tensor_mul(gt1[:], gt1[:], r_sb[:])
            nc.vector.tensor_add(gt1[:], gt1[:], ps_gin[:])
            nc.vector.tensor_add(gt1[:], gt1[:], cT_sb[:, 2, :])
            nc.scalar.activation(n_sb[:], gt1[:], AF.Tanh)
            # h_new = n + z * (h_own - n)
            nc.vector.tensor_sub(gt1[:], h_own[:], n_sb[:])
            nc.vector.tensor_mul(gt1[:], gt1[:], z_sb[:])
            nc.vector.tensor_add(hnew[:], gt1[:], n_sb[:])
            nc.vector.tensor_copy(h_own[:], hnew[:])

            # ===== AllGather h chunks =====
            w2 = nc.sync.dma_start(ag2_in[db][:], hnew[:])
            cc2 = nc.gpsimd.collective_compute(
                "AllGather", ALU.bypass,
                replica_groups=[list(range(NCORES))],
                ins=[ag2_in[db][:]], outs=[ag2_out[db][:]],
            )
            add_dep_helper(cc2.ins, w2.ins, True, "ag2 after input write")
            if prev_hT_read[db] is not None:
                add_dep_helper(cc2.ins, prev_hT_read[db].ins, True, "ag2 WAR")
            if t < steps - 1:
                r2 = nc.sync.dma_start(
                    hT[:],
                    bass.AP(ag2_out[db], 0, [[64, 128], [8192, 8], [1, 64]]),
                )
                add_dep_helper(r2.ins, cc2.ins, True, "hT read after ag2")
                prev_hT_read[db] = r2
                nc.vector.tensor_copy(hT_bf[:], hT[:])
                # rebuild h_aug (batch-major h) via PE transposes
                for k in range(NK):
                    ps_h = ps_h0 if k < 4 else ps_h1
                    kk = k % 4
                    nc.tensor.transpose(ps_h[:, kk * 128:(kk + 1) * 128],
                                        hT[:, k, :], ident128[:])
                    nc.scalar.copy(h_aug[:, k * 128:(k + 1) * 128],
                                   ps_h[:, kk * 128:(kk + 1) * 128])

        nc.sync.dma_start(out[:], toks[:])

    nc.compile()
    return nc


def prep_inputs(M, questions, word_embedding, W_out, b_out, W_ih, W_hh, b_ih, b_hh):
    """Host-side shard prep. All args np.float32 arrays."""
    f32 = np.float32
    M = np.asarray(M, f32); questions = np.asarray(questions, f32)
    word_embedding = np.ascontiguousarray(np.asarray(word_embedding, f32))
    W_out = np.asarray(W_out, f32); b_out = np.asarray(b_out, f32)
    W_ih = np.asarray(W_ih, f32); W_hh = np.asarray(W_hh, f32)
    b_ih = np.asarray(b_ih, f32); b_hh = np.asarray(b_hh, f32)

    W_pad = np.zeros((VPAD, H), f32)
    W_pad[:V] = W_out
    b_pad = np.full((VPAD,), PAD_BIAS, f32)
    b_pad[:V] = b_out

    h0 = M[:, 0, :]                      # [64, 1024]
    q = questions[:, 0, :]               # [64, 1024]
    qW = (q.astype(np.float64) @ W_ih[:, 1024:].astype(np.float64).T).astype(f32)  # [64, 3072]

    hT0 = np.ascontiguousarray(h0.T)     # [1024, 64]
    hT0_in = hT0.reshape(NK, 128, 64).transpose(1, 0, 2)  # [128, NK, 64]
    haug0 = np.zeros((64, 1032), f32)
    haug0[:, :1024] = h0
    haug0[:, 1024] = 1.0

    in_maps = []
    for c in range(NCORES):
        rows = slice(c * VSH, (c + 1) * VSH)
        Wc = W_pad[rows]                                  # [6400, 1024]
        # wt_bf [128, NK, VSH]: [p, k, v] = Wc[v, k*128+p]
        wt = Wc.T.reshape(NK, 128, VSH)                   # [k, p, v] = Wc[v, k*128+p]
        wt_bf = np.ascontiguousarray(wt.transpose(1, 0, 2)).astype(ml_dtypes.bfloat16)
        bias_bf = b_pad[rows].reshape(1, VSH).astype(ml_dtypes.bfloat16)
        w_aug = np.zeros((VSH, 1032), f32)
        w_aug[:, :1024] = Wc
        w_aug[:, 1024] = b_pad[rows]

        gr = slice(c * 128, (c + 1) * 128)
        # We rows for gates r/z/n: W_ih[g*1024 + gr, :1024]
        we = np.stack([W_ih[g * 1024 + c * 128: g * 1024 + (c + 1) * 128, :1024] for g in range(3)])   # [3, 128m, 1024]
        # we_lhsT [128p, 3, NK, 128m] = we[g, m, k*128+p]
        we_lhsT = np.ascontiguousarray(we.reshape(3, 128, NK, 128).transpose(3, 0, 2, 1))
        whh = np.stack([W_hh[g * 1024 + c * 128: g * 1024 + (c + 1) * 128, :] for g in range(3)])
        whh_lhsT = np.ascontiguousarray(whh.reshape(3, 128, NK, 128).transpose(3, 0, 2, 1))

        # cT [128p, 3, 64b]
        cT = np.zeros((128, 3, 64), f32)
        for g in range(3):
            const = qW[:, g * 1024 + c * 128: g * 1024 + (c + 1) * 128] + b_ih[g * 1024 + gr.start: g * 1024 + gr.stop]
            if g < 2:
                const = const + b_hh[g * 1024 + gr.start: g * 1024 + gr.stop]
            cT[:, g, :] = const.T
        bhh_n = b_hh[2048 + gr.start: 2048 + gr.stop].reshape(128, 1)

        h0_own = np.ascontiguousarray(h0[:, gr].T)        # [128, 64]
        coff = np.full((64, 1), c * VSH, f32)

        in_maps.append({
            "wt_bf": wt_bf,
            "bias_bf": bias_bf,
            "w_aug": w_aug,
            "wemb": word_embedding,
            "we_lhsT": we_lhsT,
            "whh_lhsT": whh_lhsT,
            "cT_in": cT,
            "bhh_n_in": bhh_n,
            "h0_own_in": h0_own,
            "hT0_in": np.ascontiguousarray(hT0_in),
            "haug0_in": haug0,
            "coff_in": coff,
        })
    return in_maps


_NC_CACHE = {}


def kernel(**inputs):
    from concourse.bass_utils import run_bass_kernel_spmd

    in_maps = prep_inputs(**inputs)
    if T not in _NC_CACHE:
        _NC_CACHE[T] = build(T)
    nc = _NC_CACHE[T]
    res = run_bass_kernel_spmd(nc, in_maps, list(range(NCORES)))
    return np.asarray(res.results[0]["out"], dtype=np.int32)


# revision 9
# speedup vs baseline: 1.2508x; 1.2508x over previous
"""Trainium2 Bass kernel for greedy GRU decode (AnswerModule).

B=64, H=1024, V=50257 (padded 51200), T=20 steps, 8 NeuronCores.

Strategy (tensor-parallel over vocab):
 - W_out/b_out/word-emb-gather sharded over vocab (6400 rows/core).
 - Screen: bf16 matmul h @ W_out_shard.T (+bias via K=1 matmul row) -> fp32 psum.
 - top-8 via max8/max_index; top-4 rescored exactly in fp32 via indirect-DMA
   gather of [W|b] rows + tensor_tensor_reduce dots.
 - AllGather (val,idx) -> global argmax with lowest-index tie-break.
 - Embedding gather from a replicated table; GRU sharded over H (128 rows/core)
   in fp32; AllGather h chunks.
"""
import sys
import numpy as np

sys.path.insert(0, "/opt/trn_rl_repo")
sys.path.insert(0, "/root/.axon_site")

import ml_dtypes

B = 64
H = 1024
V = 50257
VPAD = 51200
VSH = VPAD // 8          # 6400
T = 20
NCORES = 8
NK = H // 128            # 8 contraction chunks
NV = VSH // 512          # 12.5 -> handle as 12 full + 1 half? use 400-col tiles instead
# use vtile size 512 with 12 full tiles + 1 tile of 256: 12*512+256 = 6400
VT_SIZES = [512] * 12 + [256]
KCAND = 4
BIG = float(1 << 24)
PAD_BIAS = -10000.0


def build(steps=T):
    import concourse.bass as bass
    import concourse.bacc as bacc
    import concourse.mybir as mybir
    from concourse import tile
    from concourse.tile_rust import add_dep_helper
    from concourse.masks import make_identity

    F32 = mybir.dt.float32
    BF16 = mybir.dt.bfloat16
    U32 = mybir.dt.uint32
    I32 = mybir.dt.int32
    AF = mybir.ActivationFunctionType
    ALU = mybir.AluOpType
    AX = mybir.AxisListType

    nc = bacc.Bacc("TRN2", target_bir_lowering=False, debug=False, num_devices=NCORES)

    # ---- external inputs (per-core shards prepared on host) ----
    wt_bf = nc.dram_tensor("wt_bf", [128, NK, VSH], BF16, kind="ExternalInput")
    bias_bf = nc.dram_tensor("bias_bf", [1, VSH], BF16, kind="ExternalInput")
    w_aug = nc.dram_tensor("w_aug", [VSH, 1032], F32, kind="ExternalInput")
    wemb = nc.dram_tensor("wemb", [V, 1024], F32, kind="ExternalInput")
    we_lhsT = nc.dram_tensor("we_lhsT", [128, 3, NK, 128], F32, kind="ExternalInput")
    whh_lhsT = nc.dram_tensor("whh_lhsT", [128, 3, NK, 128], F32, kind="ExternalInput")
    cT_in = nc.dram_tensor("cT_in", [128, 3, 64], F32, kind="ExternalInput")
    bhh_n_in = nc.dram_tensor("bhh_n_in", [128, 1], F32, kind="ExternalInput")
    h0_own_in = nc.dram_tensor("h0_own_in", [128, 64], F32, kind="ExternalInput")
    hT0_in = nc.dram_tensor("hT0_in", [128, NK, 64], F32, kind="ExternalInput")
    haug0_in = nc.dram_tensor("haug0_in", [64, 1032], F32, kind="ExternalInput")
    coff_in = nc.dram_tensor("coff_in", [64, 1], F32, kind="ExternalInput")

    out = nc.dram_tensor("out", [64, steps], I32, kind="ExternalOutput")

    # ---- collective DRAM buffers (double buffered) ----
    ag1_in = [nc.dram_tensor(f"ag1_in{i}", [64, 2], F32) for i in range(2)]
    ag1_out = [nc.dram_tensor(f"ag1_out{i}", [8, 64, 2], F32, addr_space="Shared") for i in range(2)]
    ag2_in = [nc.dram_tensor(f"ag2_in{i}", [128, 64], F32) for i in range(2)]
    ag2_out = [nc.dram_tensor(f"ag2_out{i}", [8, 128, 64], F32, addr_space="Shared") for i in range(2)]

    from contextlib import ExitStack
    ctx = ExitStack()
    with ctx:
        tc = ctx.enter_context(tile.TileContext(nc))

        # ---- sbuf tensors ----
        wt_sb = nc.alloc_sbuf_tensor("wt_sb", [128, NK, VSH], BF16)
        bias_sb = nc.alloc_sbuf_tensor("bias_sb", [1, VSH], BF16)
        ones_sb = nc.alloc_sbuf_tensor("ones_sb", [1, 64], BF16)
        we_sb = nc.alloc_sbuf_tensor("we_sb", [128, 3, NK, 128], F32)
        whh_sb = nc.alloc_sbuf_tensor("whh_sb", [128, 3, NK, 128], F32)
        cT_sb = nc.alloc_sbuf_tensor("cT_sb", [128, 3, 64], F32)
        bhhn_sb = nc.alloc_sbuf_tensor("bhhn_sb", [128, 1], F32)
        coff_sb = nc.alloc_sbuf_tensor("coff_sb", [64, 1], F32)
        ident64 = nc.alloc_sbuf_tensor("ident64", [64, 64], F32)
        ident128 = nc.alloc_sbuf_tensor("ident128", [128, 128], F32)

        hT = nc.alloc_sbuf_tensor("hT", [128, NK, 64], F32)
        hT_bf = nc.alloc_sbuf_tensor("hT_bf", [128, NK, 64], BF16)
        h_aug = nc.alloc_sbuf_tensor("h_aug", [64, 1032], F32)
        h_own = nc.alloc_sbuf_tensor("h_own", [128, 64], F32)
        hnew = nc.alloc_sbuf_tensor("hnew", [128, 64], F32)
        embT = nc.alloc_sbuf_tensor("embT", [128, NK, 64], F32)
        emb_sb = nc.alloc_sbuf_tensor("emb_sb", [64, 1024], F32)

        logits = nc.alloc_sbuf_tensor("logits", [64, VSH], F32)
        maxv = nc.alloc_sbuf_tensor("maxv", [64, 8], F32)
        maxi = nc.alloc_sbuf_tensor("maxi", [64, 8], U32)
        maxi_f = nc.alloc_sbuf_tensor("maxi_f", [64, KCAND], F32)
        g4 = nc.alloc_sbuf_tensor("g4", [64, KCAND, 1032], F32)
        resc = nc.alloc_sbuf_tensor("resc", [64, KCAND], F32)

        rmax = nc.alloc_sbuf_tensor("rmax", [64, 1], F32)
        rtmp = nc.alloc_sbuf_tensor("rtmp", [64, KCAND], F32)
        rmask = nc.alloc_sbuf_tensor("rmask", [64, KCAND], F32)
        lidx = nc.alloc_sbuf_tensor("lidx", [64, 1], F32)
        agin_sb = nc.alloc_sbuf_tensor("agin_sb", [64, 2], F32)
        gg = nc.alloc_sbuf_tensor("gg", [64, 8, 2], F32)
        gmax = nc.alloc_sbuf_tensor("gmax", [64, 1], F32)
        gmask = nc.alloc_sbuf_tensor("gmask", [64, 8], F32)
        gtmp = nc.alloc_sbuf_tensor("gtmp", [64, 8], F32)
        tokf = nc.alloc_sbuf_tensor("tokf", [64, 1], F32)
        toku = nc.alloc_sbuf_tensor("toku", [64, 1], U32)
        toks = nc.alloc_sbuf_tensor("toks", [64, steps], I32)

        r_sb = nc.alloc_sbuf_tensor("r_sb", [128, 64], F32)
        z_sb = nc.alloc_sbuf_tensor("z_sb", [128, 64], F32)
        n_sb = nc.alloc_sbuf_tensor("n_sb", [128, 64], F32)
        gt1 = nc.alloc_sbuf_tensor("gt1", [128, 64], F32)
        gt2 = nc.alloc_sbuf_tensor("gt2", [128, 64], F32)

        # ---- psum ----
        ps_scr = [ctx.enter_context(nc.psum_tensor(f"ps_scr{i}", [64, 512], F32)) for i in range(2)]
        ps_g = ctx.enter_context(nc.psum_tensor("ps_g", [128, 2, 64], F32))
        ps_ghn = ctx.enter_context(nc.psum_tensor("ps_ghn", [128, 64], F32))
        ps_gin = ctx.enter_context(nc.psum_tensor("ps_gin", [128, 64], F32))
        ps_e = ctx.enter_context(nc.psum_tensor("ps_e", [128, 512], F32))
        ps_h0 = ctx.enter_context(nc.psum_tensor("ps_h0", [64, 512], F32))
        ps_h1 = ctx.enter_context(nc.psum_tensor("ps_h1", [64, 512], F32))

        # ---- preamble: load everything ----
        nc.sync.dma_start(wt_sb[:], wt_bf[:])
        nc.sync.dma_start(bias_sb[:], bias_bf[:])
        nc.sync.dma_start(we_sb[:], we_lhsT[:])
        nc.sync.dma_start(whh_sb[:], whh_lhsT[:])
        nc.sync.dma_start(cT_sb[:], cT_in[:])
        nc.sync.dma_start(bhhn_sb[:], bhh_n_in[:])
        nc.sync.dma_start(coff_sb[:], coff_in[:])
        nc.sync.dma_start(h_own[:], h0_own_in[:])
        nc.sync.dma_start(hT[:], hT0_in[:])
        nc.sync.dma_start(h_aug[:], haug0_in[:])
        nc.vector.tensor_copy(hT_bf[:], hT[:])
        nc.vector.memset(ones_sb[:], 1.0)
        make_identity(nc, ident64[:])
        make_identity(nc, ident128[:])

        prev_gg_read = [None, None]   # for WAR dep two steps back (ag1)
        prev_hT_read = [None, None]   # (ag2)

        for t in range(steps):
            db = t % 2

            # ===== screen matmuls (bf16) + bias row =====
            voff = 0
            for vt, vsz in enumerate(VT_SIZES):
                ps = ps_scr[vt % 2]
                for k in range(NK):
                    nc.tensor.matmul(
                        ps[:, 0:vsz],
                        hT_bf[:, k, :],
                        wt_sb[:, k, voff:voff + vsz],
                        start=(k == 0), stop=False)
                nc.tensor.matmul(
                    ps[:, 0:vsz],
                    ones_sb[:],
                    bias_sb[:, voff:voff + vsz],
                    start=False, stop=True)
                nc.scalar.copy(logits[:, voff:voff + vsz], ps[:, 0:vsz])
                voff += vsz

            # ===== GRU h-side matmuls (only need hT) — emitted early so the
            # TensorEngine stays busy during the argmax/AllGather window =====
            for g in range(2):
                for k in range(NK):
                    nc.tensor.matmul(
                        ps_g[:, g, :], whh_sb[:, g, k, :], hT[:, k, :],
                        start=(g == 0 and k == 0), stop=False)
            for k in range(NK):
                nc.tensor.matmul(
                    ps_ghn[:], whh_sb[:, 2, k, :], hT[:, k, :],
                    start=(k == 0), stop=(k == NK - 1))

            # ===== local top-8 =====
            nc.vector.max(out=maxv[:], in_=logits[:])
            nc.vector.max_index(out=maxi[:], in_max=maxv[:], in_values=logits[:])
            nc.vector.tensor_copy(maxi_f[:], maxi[:, 0:KCAND])

            # ===== gather candidate [W|b] rows + exact rescore =====
            for j in range(KCAND):
                nc.gpsimd.indirect_dma_start(
                    out=g4[:, j, :],
                    out_offset=None,
                    in_=w_aug[:],
                    in_offset=bass.IndirectOffsetOnAxis(ap=maxi[:, j:j + 1], axis=0),
                )
            nc.vector.tensor_mul(
                g4[:], g4[:],
                h_aug[:].unsqueeze(1).to_broadcast([64, KCAND, 1032]))
            nc.vector.tensor_reduce(resc[:], g4[:], axis=AX.X, op=ALU.add)

            # ===== local argmax of rescored (lowest global idx on ties) =====
            nc.vector.tensor_reduce(rmax[:], resc[:], axis=AX.X, op=ALU.max)
            nc.vector.tensor_scalar(rmask[:], resc[:], rmax[:, 0:1], None, op0=ALU.is_equal)
            nc.vector.tensor_scalar_add(rtmp[:], maxi_f[:], coff_sb[:, 0:1])   # global idx
            nc.vector.tensor_scalar_add(rtmp[:], rtmp[:], -BIG)
            nc.vector.tensor_mul(rtmp[:], rtmp[:], rmask[:])
            nc.vector.tensor_scalar_add(rtmp[:], rtmp[:], BIG)
            nc.vector.tensor_reduce(lidx[:], rtmp[:], axis=AX.X, op=ALU.min)
            nc.vector.tensor_copy(agin_sb[:, 0:1], rmax[:])
            nc.vector.tensor_copy(agin_sb[:, 1:2], lidx[:])

            # ===== AllGather candidates =====
            w1 = nc.sync.dma_start(ag1_in[db][:], agin_sb[:])
            cc1 = nc.gpsimd.collective_compute(
                "AllGather", ALU.bypass,
                replica_groups=[list(range(NCORES))],
                ins=[ag1_in[db][:]], outs=[ag1_out[db][:]],
            )
            add_dep_helper(cc1.ins, w1.ins, True, "ag1 after input write")
            if prev_gg_read[db] is not None:
                add_dep_helper(cc1.ins, prev_gg_read[db].ins, True, "ag1 WAR")
            r1 = nc.sync.dma_start(
                gg[:],
                bass.AP(ag1_out[db], 0, [[2, 64], [128, 8], [1, 2]]),
            )
            add_dep_helper(r1.ins, cc1.ins, True, "gg read after ag1")
            prev_gg_read[db] = r1

            # ===== global argmax combine =====
            nc.vector.tensor_reduce(gmax[:], gg[:, :, 0], axis=AX.X, op=ALU.max)
            nc.vector.tensor_scalar(gmask[:], gg[:, :, 0], gmax[:, 0:1], None, op0=ALU.is_equal)
            nc.vector.tensor_scalar_add(gtmp[:], gg[:, :, 1], -BIG)
            nc.vector.tensor_mul(gtmp[:], gtmp[:], gmask[:])
            nc.vector.tensor_scalar_add(gtmp[:], gtmp[:], BIG)
            nc.vector.tensor_reduce(tokf[:], gtmp[:], axis=AX.X, op=ALU.min)
            nc.vector.tensor_copy(toku[:], tokf[:])
            nc.vector.tensor_copy(toks[:, t:t + 1], tokf[:])

            # ===== embedding gather + transpose =====
            nc.gpsimd.indirect_dma_start(
                out=emb_sb[:],
                out_offset=None,
                in_=wemb[:],
                in_offset=bass.IndirectOffsetOnAxis(ap=toku[:, 0:1], axis=0),
            )
            for k in range(NK):
                nc.tensor.transpose(ps_e[:, k * 64:(k + 1) * 64],
                                    emb_sb[:, k * 128:(k + 1) * 128], ident64[:])
                nc.scalar.copy(embT[:, k, :], ps_e[:, k * 64:(k + 1) * 64])

            # ===== GRU emb-side matmuls (gh side was issued just after the
            # screen; these join the same psum accumulation groups) =====
            for g in range(2):
                for k in range(NK):
                    nc.tensor.matmul(
                        ps_g[:, g, :], we_sb[:, g, k, :], embT[:, k, :],
                        start=False, stop=(g == 1 and k == NK - 1))
            for k in range(NK):
                nc.tensor.matmul(
                    ps_gin[:], we_sb[:, 2, k, :], embT[:, k, :],
                    start=(k == 0), stop=(k == NK - 1))

            # ===== gates =====
            # r = sigmoid(gi_r + gh_r + c_r)  via exp/recip
            nc.vector.tensor_add(gt1[:], ps_g[:, 0, :], cT_sb[:, 0, :])
            nc.scalar.activation(gt2[:], gt1[:], AF.Exp, scale=-1.0)
            nc.vector.tensor_scalar_add(gt2[:], gt2[:], 1.0)
            nc.vector.reciprocal(r_sb[:], gt2[:])
            # z
            nc.vector.tensor_add(gt1[:], ps_g[:, 1, :], cT_sb[:, 1, :])
            nc.scalar.activation(gt2[:], gt1[:], AF.Exp, scale=-1.0)
            nc.vector.tensor_scalar_add(gt2[:], gt2[:], 1.0)
            nc.vector.reciprocal(z_sb[:], gt2[:])
            # n = tanh(gi_n + c_n + r * (gh_n + bhh_n))
            nc.vector.tensor_scalar_add(gt1[:], ps_ghn[:], bhhn_sb[:, 0:1])
            nc.vector.tensor_mul(gt1[:], gt1[:], r_sb[:])
            nc.vector.tensor_add(gt1[:], gt1[:], ps_gin[:])
            nc.vector.tensor_add(gt1[:], gt1[:], cT_sb[:, 2, :])
            nc.scalar.activation(n_sb[:], gt1[:], AF.Tanh)
            # h_new = n + z * (h_own - n)
            nc.vector.tensor_sub(gt1[:], h_own[:], n_sb[:])
            nc.vector.tensor_mul(gt1[:], gt1[:], z_sb[:])
            nc.vector.tensor_add(hnew[:], gt1[:], n_sb[:])
            nc.vector.tensor_copy(h_own[:], hnew[:])

            # ===== AllGather h chunks =====
            w2 = nc.sync.dma_start(ag2_in[db][:], hnew[:])
            cc2 = nc.gpsimd.collective_compute(
                "AllGather", ALU.bypass,
                replica_groups=[list(range(NCORES))],
                ins=[ag2_in[db][:]], outs=[ag2_out[db][:]],
            )
            add_dep_helper(cc2.ins, w2.ins, True, "ag2 after input write")
            if prev_hT_read[db] is not None:
                add_dep_helper(cc2.ins, prev_hT_read[db].ins, True, "ag2 WAR")
            if t < steps - 1:
                r2 = nc.sync.dma_start(
                    hT[:],
                    bass.AP(ag2_out[db], 0, [[64, 128], [8192, 8], [1, 64]]),
                )
                add_dep_helper(r2.ins, cc2.ins, True, "hT read after ag2")
                prev_hT_read[db] = r2
                nc.vector.tensor_copy(hT_bf[:], hT[:])
                # rebuild h_aug (batch-major h) via PE transposes
                for k in range(NK):
                    ps_h = ps_h0 if k < 4 else ps_h1
                    kk = k % 4
                    nc.tensor.transpose(ps_h[:, kk * 128:(kk + 1) * 128],
                                        hT[:, k, :], ident128[:])
                    nc.scalar.copy(h_aug[:, k * 128:(k + 1) * 128],
                                   ps_h[:, kk * 128:(kk + 1) * 128])

        nc.sync.dma_start(out[:], toks[:])

    nc.compile()
    return nc


def prep_inputs(M, questions, word_embedding, W_out, b_out, W_ih, W_hh, b_ih, b_hh):
    """Host-side shard prep. All args np.float32 arrays."""
    f32 = np.float32
    M = np.asarray(M, f32); questions = np.asarray(questions, f32)
    word_embedding = np.ascontiguousarray(np.asarray(word_embedding, f32))
    W_out = np.asarray(W_out, f32); b_out = np.asarray(b_out, f32)
    W_ih = np.asarray(W_ih, f32); W_hh = np.asarray(W_hh, f32)
    b_ih = np.asarray(b_ih, f32); b_hh = np.asarray(b_hh, f32)

    W_pad = np.zeros((VPAD, H), f32)
    W_pad[:V] = W_out
    b_pad = np.full((VPAD,), PAD_BIAS, f32)
    b_pad[:V] = b_out

    h0 = M[:, 0, :]                      # [64, 1024]
    q = questions[:, 0, :]               # [64, 1024]
    qW = (q.astype(np.float64) @ W_ih[:, 1024:].astype(np.float64).T).astype(f32)  # [64, 3072]

    hT0 = np.ascontiguousarray(h0.T)     # [1024, 64]
    hT0_in = hT0.reshape(NK, 128, 64).transpose(1, 0, 2)  # [128, NK, 64]
    haug0 = np.zeros((64, 1032), f32)
    haug0[:, :1024] = h0
    haug0[:, 1024] = 1.0

    in_maps = []
    for c in range(NCORES):
        rows = slice(c * VSH, (c + 1) * VSH)
        Wc = W_pad[rows]                                  # [6400, 1024]
        # wt_bf [128, NK, VSH]: [p, k, v] = Wc[v, k*128+p]
        wt = Wc.T.reshape(NK, 128, VSH)                   # [k, p, v] = Wc[v, k*128+p]
        wt_bf = np.ascontiguousarray(wt.transpose(1, 0, 2)).astype(ml_dtypes.bfloat16)
        bias_bf = b_pad[rows].reshape(1, VSH).astype(ml_dtypes.bfloat16)
        w_aug = np.zeros((VSH, 1032), f32)
        w_aug[:, :1024] = Wc
        w_aug[:, 1024] = b_pad[rows]

        gr = slice(c * 128, (c + 1) * 128)
        # We rows for gates r/z/n: W_ih[g*1024 + gr, :1024]
        we = np.stack([W_ih[g * 1024 + c * 128: g * 1024 + (c + 1) * 128, :1024] for g in range(3)])   # [3, 128m, 1024]
        # we_lhsT [128p, 3, NK, 128m] = we[g, m, k*128+p]
        we_lhsT = np.ascontiguousarray(we.reshape(3, 128, NK, 128).transpose(3, 0, 2, 1))
        whh = np.stack([W_hh[g * 1024 + c * 128: g * 1024 + (c + 1) * 128, :] for g in range(3)])
        whh_lhsT = np.ascontiguousarray(whh.reshape(3, 128, NK, 128).transpose(3, 0, 2, 1))

        # cT [128p, 3, 64b]
        cT = np.zeros((128, 3, 64), f32)
        for g in range(3):
            const = qW[:, g * 1024 + c * 128: g * 1024 + (c + 1) * 128] + b_ih[g * 1024 + gr.start: g * 1024 + gr.stop]
            if g < 2:
                const = const + b_hh[g * 1024 + gr.start: g * 1024 + gr.stop]
            cT[:, g, :] = const.T
        bhh_n = b_hh[2048 + gr.start: 2048 + gr.stop].reshape(128, 1)

        h0_own = np.ascontiguousarray(h0[:, gr].T)        # [128, 64]
        coff = np.full((64, 1), c * VSH, f32)

        in_maps.append({
            "wt_bf": wt_bf,
            "bias_bf": bias_bf,
            "w_aug": w_aug,
            "wemb": word_embedding,
            "we_lhsT": we_lhsT,
            "whh_lhsT": whh_lhsT,
            "cT_in": cT,
            "bhh_n_in": bhh_n,
            "h0_own_in": h0_own,
            "hT0_in": np.ascontiguousarray(hT0_in),
            "haug0_in": haug0,
            "coff_in": coff,
        })
    return in_maps


_NC_CACHE = {}


def kernel(**inputs):
    from concourse.bass_utils import run_bass_kernel_spmd

    in_maps = prep_inputs(**inputs)
    if T not in _NC_CACHE:
        _NC_CACHE[T] = build(T)
    nc = _NC_CACHE[T]
    res = run_bass_kernel_spmd(nc, in_maps, list(range(NCORES)))
    return np.asarray(res.results[0]["out"], dtype=np.int32)
